# revision 18
# baseline (speedup 1.0000x reference)
"""Trainium2 Bass kernel v5: ragged GQA flash-decode attention.

Numerics (v3): K/Q/P bf16, V fp8 e3m4 (budget 2e-2; measures 1.679e-2).
Q (and P^T) are PE-stationary (4-col weight loads); K^T / V stream as the
moving operands. Scores+PV col-tile 4-way across PE quadrants
(tile_position); output strips must start 32-aligned (walrus). Host zeroes
invalid K/V slots (p=exp(0)=1 lands in l, 0 in O) and subtracts the exact
count from l. ACT exp's accum_out produces l for free.

Pipeline (v4, HW-trace driven):
  - For_i's per-iteration all-engine barrier costs ~14us, so `unroll`
    iterations are emitted per trip; across each barrier the pre-barrier
    body prefetches the next body's first kv groups into a fixed kv ring
    (streams during the compute tail) and defers its last wb chunk past
    the barrier (the next body flushes it).
  - qt loads once, outside the loop. The sync HWDGE queue carries ONLY kv
    loads; o/l writebacks ride gpsimd/SWDGE.
  - o is [16, 256*TGT] (real rows only); l is a separate 128-partition f32
    tensor so its writeback spreads across all SBUF AXI ports.

Packing (v5): no pair padding — sequences with an odd tile count get
`floor(tiles/2)` 2-tile groups plus ONE 1-tile group (second program phase),
cutting DMA bytes ~2.5%. Per core: TG2 pair-groups then TG1 singles.

Layouts (per core):
  kv2d [TG2*128, 3072] bf16-typed: pair-group g row p, col j*1536 + c;
       tile j: K^T bf16 at c in [0,1024) ([d=p, h*128+s]), V fp8e3m4
       (bitcast, bf16-col units) at c in [1024,1536) ([s=p, 64h + d/2]).
  kv1d [TG1*128, 1536]: same single-tile layout.
  qt   [128, 32*TGT] bf16: q for group g at cols 32g..32g+32, col = h*G+g.
  o    [16, 256*TGT] bf16: group g cols 256g+128*b2: head h=4*b2+c at row
       strip 4c..4c+4 (rows=[G]), cols = d.
  lout [128, 2*TGT] f32: accum_out sums; row 32c+g' (head 4*b2+c member g')
       at col 2g+b2.
"""

import math
from contextlib import ExitStack

import numpy as np

N_CORES = 8
B, HQ, HKV, D = 16, 32, 8, 128
G = HQ // HKV
ROW = 2 * HKV * D   # 2048 floats per kv_buffer row
TW = 1536           # bf16 cols per tile block (K 1024 + V-as-fp8 512)
GW = 2 * TW
SCALE = 1.0 / math.sqrt(D)

_COMPILED: dict = {}


def _build_program(TG2: int, TG1: int, niter: int = 1, *, kv_bufs=12,
                   psum_bufs=2, p_bufs=4, skew=1, wb_chunk=5, unroll=8,
                   prefetch=2, staggered=False):
    import concourse.mybir as mybir
    import concourse.tile as tile
    from concourse import bacc
    from concourse.masks import make_identity

    f32 = mybir.dt.float32
    bf16 = mybir.dt.bfloat16
    f8e3 = mybir.dt.float8e3
    nc = bacc.Bacc("TRN2", target_bir_lowering=False, debug=False, num_devices=N_CORES)

    TGT = TG2 + TG1
    kv2d = nc.dram_tensor("kv2d", [max(TG2, 1) * 128, GW], bf16,
                          kind="ExternalInput").ap()
    kv1d = None
    if TG1:
        kv1d = nc.dram_tensor("kv1d", [TG1 * 128, TW], bf16,
                              kind="ExternalInput").ap()
    qt = nc.dram_tensor("qt", [128, 32 * TGT], bf16, kind="ExternalInput").ap()
    o = nc.dram_tensor("o", [16, 256 * TGT], bf16, kind="ExternalOutput").ap()
    lout = nc.dram_tensor("lout", [128, 2 * TGT], f32, kind="ExternalOutput").ap()

    def gsrc(g):
        """DMA source AP and tile-width w for group g."""
        if g < TG2:
            return kv2d[g * 128:(g + 1) * 128, :], 2
        t = g - TG2
        return kv1d[t * 128:(t + 1) * 128, :], 1

    with tile.TileContext(nc) as tc, ExitStack() as ctx:
        kv_pool = ctx.enter_context(tc.tile_pool(name="kv", bufs=1))
        sc_pools = [
            ctx.enter_context(tc.tile_pool(name=f"sc{i}", bufs=psum_bufs, space="PSUM"))
            for i in range(2)
        ]
        p_pools = [
            ctx.enter_context(tc.tile_pool(name=f"p{i}", bufs=p_bufs))
            for i in range(2)
        ]
        pts_pools = [
            ctx.enter_context(tc.tile_pool(name=f"pts{i}", bufs=p_bufs))
            for i in range(2)
        ]
        tp_ps_pool = ctx.enter_context(
            tc.tile_pool(name="tp", bufs=1, space="PSUM"))
        o_pool = ctx.enter_context(tc.tile_pool(name="ops", bufs=psum_bufs, space="PSUM"))
        const_pool = ctx.enter_context(tc.tile_pool(name="const", bufs=1))
        io_pool = ctx.enter_context(tc.tile_pool(name="io", bufs=1))

        o_all = io_pool.tile([128, 256 * TGT], bf16)
        l_all = io_pool.tile([128, 2 * TGT], f32)

        ident = const_pool.tile([128, 128], bf16)
        make_identity(nc, ident[:])

        # qt is loop-invariant: load it once, before the iteration loop.
        qt_s = io_pool.tile([128, 32 * TGT], bf16)
        nc.sync.dma_start(qt_s[:], qt)

        # Fixed kv ring (explicit slices, not pool-rotated) so a body can
        # prefetch the NEXT body's first groups into the exact addresses that
        # body will read (cross-barrier identity).
        kv_ring = kv_pool.tile([128, kv_bufs * GW], bf16, name="kv_ring")
        kvt = [kv_ring[:, i * GW:(i + 1) * GW] for i in range(kv_bufs)]

        def kv_dma(g):
            src, w = gsrc(g)
            nc.sync.dma_start(kvt[g % kv_bufs][:, 0:w * TW], src)

        def v_ap(kv2, j, h):
            return kv2[:, TW * j + 1024 + 64 * h:
                       TW * j + 1024 + 64 * (h + 1)].bitcast(f8e3)

        def emit_pv(st):
            g, kv2, p, w = st
            # transpose P blocks on the PE (data loads as weights, identity
            # streams through), then one DVE copy per batch to SBUF
            pts = [pts_pools[i].tile([128, 128 * w], bf16, name=f"pts_{i}")
                   for i in range(2)]
            for i in range(2):
                tp = tp_ps_pool.tile([128, 128 * w], bf16, name=f"tp_{i}")
                for j in range(w):
                    nc.tensor.transpose(tp[:, 128 * j:128 * (j + 1)],
                                        p[i][:, 128 * j:128 * (j + 1)], ident[:])
                nc.vector.tensor_copy(pts[i][:], tp[:])
            O = o_pool.tile([128, 256], f32)
            for b2 in range(2):
                for j in range(w):
                    for c in range(4):
                        h = 4 * b2 + c
                        nc.tensor.matmul(
                            O[32 * c:32 * c + 4, 128 * b2:128 * b2 + 128],
                            pts[b2][:, 128 * j + 32 * c:128 * j + 32 * c + 4],
                            v_ap(kv2, j, h),
                            start=(j == 0), stop=(j == w - 1),
                            tile_position=(0, 32 * c),
                        )
            nc.vector.tensor_copy(o_all[:, 256 * g:256 * g + 256], O[:])
            if ((g + 1) % wb_chunk == 0 and g != TGT - 1) or \
                    (g == TGT - 1 and not emit_pv.defer_tail):
                wb_flush(emit_pv.wb_done, g + 1)
                emit_pv.wb_done = g + 1

        def wb_flush(g0, g1):
            c0, c1 = 256 * g0, 256 * g1
            for a in range(4):
                nc.gpsimd.dma_start(o[4 * a:4 * (a + 1), c0:c1],
                                    o_all[32 * a:32 * a + 4, c0:c1])
            nc.gpsimd.dma_start(lout[:, 2 * g0:2 * g1], l_all[:, 2 * g0:2 * g1])

        last_wb = TGT - ((TGT - 1) % wb_chunk + 1)  # start of the tail wb chunk

        def body(skip_pf=0, tail_pf=0, defer_tail=False, flush_prev=False):
            """One iteration. skip_pf: first N groups' kv DMAs were issued by
            the previous body (cross-barrier prefetch). tail_pf: issue the
            next body's first N kv DMAs after ours. defer_tail: leave the
            last wb chunk pending (the next body or the epilogue flushes it).
            flush_prev: write back the previous body's pending tail chunk."""
            emit_pv.wb_done = 0
            emit_pv.defer_tail = defer_tail
            if flush_prev:
                wb_flush(last_wb, TGT)
            pend = []
            for g in range(TGT):
                w = 2 if g < TG2 else 1
                kv2 = kvt[g % kv_bufs]
                if g >= skip_pf:
                    kv_dma(g)
                if g == TGT - 1:
                    for gp in range(tail_pf):
                        kv_dma(gp)
                kv2r = kv2[:, 0:w * TW].rearrange("p (j c) -> p j c", j=w)
                sc = [sc_pools[i].tile([128, 128 * w], f32, name=f"sc_{i}")
                      for i in range(2)]
                p = [p_pools[i].tile([128, 128 * w], bf16, name=f"p_{i}")
                     for i in range(2)]
                lg = l_all[:, 2 * g:2 * (g + 1)]
                for b2 in range(2):
                    for c in range(4):
                        h = 4 * b2 + c
                        nc.tensor.matmul(
                            sc[b2][32 * c:32 * c + 4, :],
                            qt_s[:, 32 * g + 4 * h:32 * g + 4 * h + 4],
                            kv2r[:, :, 128 * h:128 * (h + 1)],
                            start=True, stop=True,
                            tile_position=(0, 32 * c),
                        )
                    # exp for batch b2 starts while batch 1-b2 scores run
                    nc.scalar.activation(
                        p[b2][:],
                        sc[b2][:],
                        mybir.ActivationFunctionType.Exp,
                        scale=SCALE,
                        accum_out=lg[:, b2:b2 + 1],
                    )
                pend.append((g, kv2, p, w))
                if len(pend) > skew:
                    emit_pv(pend.pop(0))
            for st in pend:
                emit_pv(st)

        if niter > 1:
            # Unroll `unroll` iterations per For_i trip: the loop's all-engine
            # barrier (a full pipeline drain, ~14us) is paid once per trip.
            n_loop, n_rem = divmod(niter, unroll)
            PF = min(prefetch, TG2) if n_loop else 0
            if n_loop:
                for gp in range(PF):
                    kv_dma(gp)
                with tc.For_i(0, n_loop, 1, staggered_reset=staggered):
                    for u in range(unroll):
                        body(skip_pf=PF if u == 0 else 0,
                             tail_pf=PF if u == unroll - 1 else 0,
                             defer_tail=True, flush_prev=True)
            for r in range(n_rem):
                body(skip_pf=PF if r == 0 else 0,
                     defer_tail=True, flush_prev=(n_loop > 0 or r > 0))
            wb_flush(last_wb, TGT)
        else:
            body()

    nc.compile()
    return nc


def _make_runner(nc):
    """Persistent jitted SPMD runner for a compiled Bacc program (axon path)."""
    import jax
    import concourse.mybir as mybir
    from jax.experimental.shard_map import shard_map
    from jax.sharding import Mesh, PartitionSpec

    from concourse.bass2jax import (
        _bass_exec_p,
        install_neuronx_cc_hook,
        partition_id_tensor,
    )

    install_neuronx_cc_hook()

    partition_name = nc.partition_id_tensor.name if nc.partition_id_tensor else None
    in_names, out_names, out_avals, zero_shapes = [], [], [], []
    for alloc in nc.m.functions[0].allocations:
        if not isinstance(alloc, mybir.MemoryLocationSet):
            continue
        name = alloc.memorylocations[0].name
        if alloc.kind == "ExternalInput":
            if name != partition_name:
                in_names.append(name)
        elif alloc.kind == "ExternalOutput":
            out_names.append(name)
            shape = tuple(alloc.tensor_shape)
            dtype = mybir.dt.np(alloc.dtype)
            out_avals.append(jax.core.ShapedArray(shape, dtype))
            zero_shapes.append((shape, dtype))
    n_params = len(in_names)
    n_outs = len(out_avals)
    all_in_names = list(in_names) + list(out_names)
    if partition_name is not None:
        all_in_names.append(partition_name)

    def _body(*args):
        operands = list(args)
        if partition_name is not None:
            operands.append(partition_id_tensor())
        outs = _bass_exec_p.bind(
            *operands,
            out_avals=tuple(out_avals),
            in_names=tuple(all_in_names),
            out_names=tuple(out_names),
            lowering_input_output_aliases=(),
            sim_require_finite=True,
            sim_require_nnan=True,
            nc=nc,
        )
        return tuple(outs)

    devices = jax.devices()[:N_CORES]
    assert len(devices) >= N_CORES, f"need {N_CORES} devices, have {len(devices)}"
    mesh = Mesh(np.asarray(devices[:N_CORES]), ("core",))
    in_specs = (PartitionSpec("core"),) * (n_params + n_outs)
    out_specs = (PartitionSpec("core"),) * n_outs
    donate = tuple(range(n_params, n_params + n_outs))
    sharded = jax.jit(
        shard_map(
            _body, mesh=mesh, in_specs=in_specs, out_specs=out_specs, check_rep=False
        ),
        donate_argnums=donate,
        keep_unused=True,
    )

    def run(concat_inputs):
        args = [concat_inputs[name] for name in in_names]
        zeros = [
            np.zeros((N_CORES * s[0], *s[1:]), d) for (s, d) in zero_shapes
        ]
        out_arrs = sharded(*args, *zeros)
        out_arrs = [np.asarray(a) for a in out_arrs]
        return {name: out_arrs[i] for i, name in enumerate(out_names)}

    run.in_names = in_names
    run.out_names = out_names
    run.out_avals = out_avals
    run.zero_shapes = zero_shapes
    run.sharded = sharded
    run.mesh = mesh
    return run


def _plan(b_seq_len):
    """Mixed plan: per seq, floor(tiles/2) pair-groups + (tiles%2) singles.

    Returns (groups2, groups1, TG2, TG1, n_invalid):
      groups2: [(b, jg)] padded to 8*TG2 (pair jg covers seq tiles 2jg, 2jg+1)
      groups1: [(b, jt)] padded to 8*TG1 (single covers seq tile jt)
      n_invalid[b] = ntiles_b*128 - len_b  (tile-tail padding only)
    """
    lens = [int(x) for x in b_seq_len]
    groups2, groups1 = [], []
    n_invalid = np.zeros(B, dtype=np.int64)
    for b, ln in enumerate(lens):
        ntiles = (ln + 127) // 128
        n_invalid[b] = ntiles * 128 - ln
        for jg in range(ntiles // 2):
            groups2.append((b, jg))
        if ntiles % 2:
            groups1.append((b, ntiles - 1))
    TG2 = (len(groups2) + N_CORES - 1) // N_CORES
    TG1 = (len(groups1) + N_CORES - 1) // N_CORES
    groups2 += [(-1, -1)] * (N_CORES * TG2 - len(groups2))
    groups1 += [(-1, -1)] * (N_CORES * TG1 - len(groups1))
    return groups2, groups1, TG2, TG1, n_invalid


def _pack_tiles(ent, kvb, lens, starts, csi, new_kv):
    """ent: [(b, j)] tile entries (b<0 = zero tile). Returns packed bytes
    [n, 128, 2*TW] uint8 (K^T bf16 | V fp8e3m4)."""
    import ml_dtypes

    bf = ml_dtypes.bfloat16
    e3 = ml_dtypes.float8_e3m4
    ent = np.array(ent, dtype=np.int64).reshape(-1, 2)
    n = len(ent)
    real = np.nonzero(ent[:, 0] >= 0)[0]
    eb, ej = ent[real, 0], ent[real, 1]

    rows = (starts[eb] + ej * 128)[:, None] + np.arange(128)[None, :]
    arr = kvb[rows]  # [nreal, 128, 2048] f32 gather
    nvalid = np.minimum(128, lens[eb] - ej * 128)
    mask = np.arange(128)[None, :] >= nvalid[:, None]
    arr[mask] = 0.0
    sel = np.nonzero((csi[eb] >= rows[:, 0]) & (csi[eb] < rows[:, 0] + 128))[0]
    arr[sel, (csi[eb[sel]] - rows[sel, 0])] = new_kv[eb[sel]]

    kt = (
        arr[:, :, :HKV * D].reshape(-1, 128, HKV, D)
        .transpose(0, 3, 2, 1).reshape(-1, 128, HKV * 128)
    )
    v = arr[:, :, HKV * D:]
    tiles = np.zeros((n, 128, 2 * TW), dtype=np.uint8)
    tiles[real, :, :2048] = kt.astype(bf).view(np.uint8)
    tiles[real, :, 2048:3072] = v.astype(e3).view(np.uint8)
    return tiles


def _pack(xq, xk, xv, kv_buffer, cur_select_index, start_index, b_seq_len,
          plan):
    import ml_dtypes

    bf = ml_dtypes.bfloat16
    groups2, groups1, TG2, TG1, _ = plan
    TGT = TG2 + TG1
    lens = np.asarray(b_seq_len, dtype=np.int64)
    starts = np.asarray(start_index, dtype=np.int64)
    csi = np.asarray(cur_select_index, dtype=np.int64)
    kvb = np.asarray(kv_buffer).reshape(-1, ROW)
    new_kv = np.concatenate(
        [np.asarray(xk)[:, 0], np.asarray(xv)[:, 0]], axis=1
    ).reshape(B, ROW)
    q_bf = np.asarray(xq)[:, 0].transpose(0, 2, 1).astype(bf)  # [B, D, HQ]

    ent2 = []
    for b, jg in groups2:
        for u in range(2):
            ent2.append((b, 2 * jg + u) if b >= 0 else (-1, -1))
    t2 = _pack_tiles(ent2, kvb, lens, starts, csi, new_kv)
    kv2_all = (
        t2.reshape(N_CORES * TG2, 2, 128, 2 * TW)
        .transpose(0, 2, 1, 3).reshape(N_CORES * TG2 * 128, 4 * TW)
        .view(bf)
    )
    out = {"kv2d": kv2_all}
    if TG1:
        t1 = _pack_tiles(groups1, kvb, lens, starts, csi, new_kv)
        out["kv1d"] = t1.reshape(N_CORES * TG1 * 128, 2 * TW).view(bf)

    qt_all = np.zeros((N_CORES, 128, 32 * TGT), dtype=bf)
    for c in range(N_CORES):
        for s in range(TG2):
            b = groups2[c * TG2 + s][0]
            if b >= 0:
                qt_all[c, :, 32 * s:32 * s + 32] = q_bf[b]
        for t in range(TG1):
            b = groups1[c * TG1 + t][0]
            if b >= 0:
                s = TG2 + t
                qt_all[c, :, 32 * s:32 * s + 32] = q_bf[b]
    out["qt"] = qt_all.reshape(N_CORES * 128, 32 * TGT)
    return out


def _combine(o_cat, l_cat, plan):
    groups2, groups1, TG2, TG1, n_invalid = plan
    TGT = TG2 + TG1
    acc = np.zeros((B, HQ, D), dtype=np.float64)
    lacc = np.zeros((B, HQ), dtype=np.float64)
    o_cat = np.asarray(o_cat).reshape(N_CORES, 16, 256 * TGT)
    l_cat = np.asarray(l_cat).reshape(N_CORES, 128, 2 * TGT)
    with np.errstate(invalid="ignore"):
        o_f = o_cat.astype(np.float64)

    def add_group(cc, s, b):
        for b2 in range(2):
            for c in range(4):
                hq = 16 * b2 + 4 * c
                Ob = o_f[cc][4 * c:4 * c + 4,
                             256 * s + 128 * b2:256 * s + 128 * (b2 + 1)]
                lb = l_cat[cc, 32 * c:32 * c + 4, 2 * s + b2]
                acc[b, hq:hq + 4] += Ob
                lacc[b, hq:hq + 4] += lb

    for i, (b, jg) in enumerate(groups2):
        if b >= 0:
            cc, s = divmod(i, TG2)
            add_group(cc, s, b)
    for i, (b, jt) in enumerate(groups1):
        if b >= 0:
            cc, t = divmod(i, TG1)
            add_group(cc, TG2 + t, b)
    lacc -= n_invalid[:, None]
    out = acc / lacc[:, :, None]
    return out.reshape(B, 1, HQ * D).astype(np.float32)


def get_compiled(TG2, TG1, niter=1):
    key = (TG2, TG1, niter)
    if key not in _COMPILED:
        nc = _build_program(TG2, TG1, niter)
        _COMPILED[key] = _make_runner(nc)
    return _COMPILED[key]


def kernel(xq, xk, xv, kv_buffer, cur_select_index, start_index, b_seq_len,
           max_actual_seq_len=None):
    plan = _plan(b_seq_len)
    inputs = _pack(xq, xk, xv, kv_buffer, cur_select_index, start_index,
                   b_seq_len, plan)
    run = get_compiled(plan[2], plan[3])
    outs = run(inputs)
    return _combine(outs["o"], outs["lout"], plan)


# revision 19
# speedup vs baseline: 1.0466x; 1.0466x over previous
"""Trainium2 Bass kernel v5: ragged GQA flash-decode attention.

Numerics (v3): K/Q/P bf16, V fp8 e3m4 (budget 2e-2; measures 1.679e-2).
Q (and P^T) are PE-stationary (4-col weight loads); K^T / V stream as the
moving operands. Scores+PV col-tile 4-way across PE quadrants
(tile_position); output strips must start 32-aligned (walrus). Host zeroes
invalid K/V slots (p=exp(0)=1 lands in l, 0 in O) and subtracts the exact
count from l. ACT exp's accum_out produces l for free.

Pipeline (v4, HW-trace driven):
  - For_i's per-iteration all-engine barrier costs ~14us, so `unroll`
    iterations are emitted per trip; across each barrier the pre-barrier
    body prefetches the next body's first kv groups into a fixed kv ring
    (streams during the compute tail) and defers its last wb chunk past
    the barrier (the next body flushes it).
  - qt loads once, outside the loop. The sync HWDGE queue carries ONLY kv
    loads; o/l writebacks ride gpsimd/SWDGE.
  - o is [16, 256*TGT] (real rows only); l is a separate 128-partition f32
    tensor so its writeback spreads across all SBUF AXI ports.

Packing (v5): no pair padding — sequences with an odd tile count get
`floor(tiles/2)` 2-tile groups plus ONE 1-tile group (second program phase),
cutting DMA bytes ~2.5%. Per core: TG2 pair-groups then TG1 singles.

Layouts (per core):
  kv2d [TG2*128, 3072] bf16-typed: pair-group g row p, col j*1536 + c;
       tile j: K^T bf16 at c in [0,1024) ([d=p, h*128+s]), V fp8e3m4
       (bitcast, bf16-col units) at c in [1024,1536) ([s=p, 64h + d/2]).
  kv1d [TG1*128, 1536]: same single-tile layout.
  qt   [128, 32*TGT] bf16: q for group g at cols 32g..32g+32, col = h*G+g.
  o    [16, 256*TGT] bf16: group g cols 256g+128*b2: head h=4*b2+c at row
       strip 4c..4c+4 (rows=[G]), cols = d.
  lout [128, 2*TGT] f32: accum_out sums; row 32c+g' (head 4*b2+c member g')
       at col 2g+b2.
"""

import math
from contextlib import ExitStack

import numpy as np

N_CORES = 8
B, HQ, HKV, D = 16, 32, 8, 128
G = HQ // HKV
ROW = 2 * HKV * D   # 2048 floats per kv_buffer row
TW = 1536           # bf16 cols per tile block (K 1024 + V-as-fp8 512)
GW = 2 * TW
SCALE = 1.0 / math.sqrt(D)

_COMPILED: dict = {}


def _build_program(TG2: int, TG1: int, niter: int = 1, *, kv_bufs=12,
                   psum_bufs=2, p_bufs=4, skew=1, wb_chunk=5, unroll=8,
                   prefetch=2, staggered=False):
    import concourse.mybir as mybir
    import concourse.tile as tile
    from concourse import bacc
    from concourse.masks import make_identity

    f32 = mybir.dt.float32
    bf16 = mybir.dt.bfloat16
    f8e3 = mybir.dt.float8e3
    nc = bacc.Bacc("TRN2", target_bir_lowering=False, debug=False, num_devices=N_CORES)

    TGT = TG2 + TG1
    kv2d = nc.dram_tensor("kv2d", [max(TG2, 1) * 128, GW], bf16,
                          kind="ExternalInput").ap()
    kv1d = None
    if TG1:
        kv1d = nc.dram_tensor("kv1d", [TG1 * 128, TW], bf16,
                              kind="ExternalInput").ap()
    qt = nc.dram_tensor("qt", [128, 32 * TGT], bf16, kind="ExternalInput").ap()
    o = nc.dram_tensor("o", [16, 256 * TGT], bf16, kind="ExternalOutput").ap()
    lout = nc.dram_tensor("lout", [128, 2 * TGT], f32, kind="ExternalOutput").ap()

    def gsrc(g):
        """DMA source AP and tile-width w for group g."""
        if g < TG2:
            return kv2d[g * 128:(g + 1) * 128, :], 2
        t = g - TG2
        return kv1d[t * 128:(t + 1) * 128, :], 1

    with tile.TileContext(nc) as tc, ExitStack() as ctx:
        kv_pool = ctx.enter_context(tc.tile_pool(name="kv", bufs=1))
        sc_pools = [
            ctx.enter_context(tc.tile_pool(name=f"sc{i}", bufs=psum_bufs, space="PSUM"))
            for i in range(2)
        ]
        p_pools = [
            ctx.enter_context(tc.tile_pool(name=f"p{i}", bufs=p_bufs))
            for i in range(2)
        ]
        pts_pools = [
            ctx.enter_context(tc.tile_pool(name=f"pts{i}", bufs=p_bufs))
            for i in range(2)
        ]
        tp_ps_pool = ctx.enter_context(
            tc.tile_pool(name="tp", bufs=1, space="PSUM"))
        o_pool = ctx.enter_context(tc.tile_pool(name="ops", bufs=psum_bufs, space="PSUM"))
        const_pool = ctx.enter_context(tc.tile_pool(name="const", bufs=1))
        io_pool = ctx.enter_context(tc.tile_pool(name="io", bufs=1))

        o_all = io_pool.tile([128, 256 * TGT], bf16)
        l_all = io_pool.tile([128, 2 * TGT], f32)

        ident = const_pool.tile([128, 128], bf16)
        make_identity(nc, ident[:])

        # qt is loop-invariant: load it once, before the iteration loop.
        qt_s = io_pool.tile([128, 32 * TGT], bf16)
        nc.sync.dma_start(qt_s[:], qt)

        # Fixed kv ring (explicit slices, not pool-rotated) so a body can
        # prefetch the NEXT body's first groups into the exact addresses that
        # body will read (cross-barrier identity).
        kv_ring = kv_pool.tile([128, kv_bufs * GW], bf16, name="kv_ring")
        kvt = [kv_ring[:, i * GW:(i + 1) * GW] for i in range(kv_bufs)]

        def kv_dma(g):
            src, w = gsrc(g)
            nc.sync.dma_start(kvt[g % kv_bufs][:, 0:w * TW], src)

        def v_ap(kv2, j, h):
            return kv2[:, TW * j + 1024 + 64 * h:
                       TW * j + 1024 + 64 * (h + 1)].bitcast(f8e3)

        def emit_pv(st):
            g, kv2, p, w = st
            # transpose P blocks on the PE (data loads as weights, identity
            # streams through), then one DVE copy per batch to SBUF
            pts = [pts_pools[i].tile([128, 128 * w], bf16, name=f"pts_{i}")
                   for i in range(2)]
            for i in range(2):
                tp = tp_ps_pool.tile([128, 128 * w], bf16, name=f"tp_{i}")
                for j in range(w):
                    nc.tensor.transpose(tp[:, 128 * j:128 * (j + 1)],
                                        p[i][:, 128 * j:128 * (j + 1)], ident[:])
                nc.vector.tensor_copy(pts[i][:], tp[:])
            O = o_pool.tile([128, 256], f32)
            for b2 in range(2):
                for j in range(w):
                    for c in range(4):
                        h = 4 * b2 + c
                        nc.tensor.matmul(
                            O[32 * c:32 * c + 4, 128 * b2:128 * b2 + 128],
                            pts[b2][:, 128 * j + 32 * c:128 * j + 32 * c + 4],
                            v_ap(kv2, j, h),
                            start=(j == 0), stop=(j == w - 1),
                            tile_position=(0, 32 * c),
                        )
            nc.vector.tensor_copy(o_all[:, 256 * g:256 * g + 256], O[:])
            if ((g + 1) % wb_chunk == 0 and g != TGT - 1) or \
                    (g == TGT - 1 and not emit_pv.defer_tail):
                wb_flush(emit_pv.wb_done, g + 1)
                emit_pv.wb_done = g + 1

        def wb_flush(g0, g1):
            c0, c1 = 256 * g0, 256 * g1
            for a in range(4):
                nc.gpsimd.dma_start(o[4 * a:4 * (a + 1), c0:c1],
                                    o_all[32 * a:32 * a + 4, c0:c1])
            nc.gpsimd.dma_start(lout[:, 2 * g0:2 * g1], l_all[:, 2 * g0:2 * g1])

        last_wb = TGT - ((TGT - 1) % wb_chunk + 1)  # start of the tail wb chunk

        def body(skip_pf=0, tail_pf=0, defer_tail=False, flush_prev=False):
            """One iteration. skip_pf: first N groups' kv DMAs were issued by
            the previous body (cross-barrier prefetch). tail_pf: issue the
            next body's first N kv DMAs after ours. defer_tail: leave the
            last wb chunk pending (the next body or the epilogue flushes it).
            flush_prev: write back the previous body's pending tail chunk."""
            emit_pv.wb_done = 0
            emit_pv.defer_tail = defer_tail
            if flush_prev:
                wb_flush(last_wb, TGT)
            pend = []
            for g in range(TGT):
                w = 2 if g < TG2 else 1
                kv2 = kvt[g % kv_bufs]
                if g >= skip_pf:
                    kv_dma(g)
                if g == TGT - 1:
                    for gp in range(tail_pf):
                        kv_dma(gp)
                kv2r = kv2[:, 0:w * TW].rearrange("p (j c) -> p j c", j=w)
                sc = [sc_pools[i].tile([128, 128 * w], f32, name=f"sc_{i}")
                      for i in range(2)]
                p = [p_pools[i].tile([128, 128 * w], bf16, name=f"p_{i}")
                     for i in range(2)]
                lg = l_all[:, 2 * g:2 * (g + 1)]
                for b2 in range(2):
                    for c in range(4):
                        h = 4 * b2 + c
                        nc.tensor.matmul(
                            sc[b2][32 * c:32 * c + 4, :],
                            qt_s[:, 32 * g + 4 * h:32 * g + 4 * h + 4],
                            kv2r[:, :, 128 * h:128 * (h + 1)],
                            start=True, stop=True,
                            tile_position=(0, 32 * c),
                        )
                    # exp for batch b2 starts while batch 1-b2 scores run
                    nc.scalar.activation(
                        p[b2][:],
                        sc[b2][:],
                        mybir.ActivationFunctionType.Exp,
                        scale=SCALE,
                        accum_out=lg[:, b2:b2 + 1],
                    )
                pend.append((g, kv2, p, w))
                if len(pend) > skew:
                    emit_pv(pend.pop(0))
            for st in pend:
                emit_pv(st)

        if niter > 1:
            # Unroll `unroll` iterations per For_i trip: the loop's all-engine
            # barrier (a full pipeline drain, ~14us) is paid once per trip.
            n_loop, n_rem = divmod(niter, unroll)
            PF = min(prefetch, TG2) if n_loop else 0
            if n_loop:
                for gp in range(PF):
                    kv_dma(gp)
                with tc.For_i(0, n_loop, 1, staggered_reset=staggered):
                    for u in range(unroll):
                        body(skip_pf=PF if u == 0 else 0,
                             tail_pf=PF if u == unroll - 1 else 0,
                             defer_tail=True, flush_prev=True)
            for r in range(n_rem):
                body(skip_pf=PF if r == 0 else 0,
                     defer_tail=True, flush_prev=(n_loop > 0 or r > 0))
            wb_flush(last_wb, TGT)
        else:
            body()

    nc.compile()
    return nc


def _make_runner(nc):
    """Persistent jitted SPMD runner for a compiled Bacc program (axon path)."""
    import jax
    import concourse.mybir as mybir
    from jax.experimental.shard_map import shard_map
    from jax.sharding import Mesh, PartitionSpec

    from concourse.bass2jax import (
        _bass_exec_p,
        install_neuronx_cc_hook,
        partition_id_tensor,
    )

    install_neuronx_cc_hook()

    partition_name = nc.partition_id_tensor.name if nc.partition_id_tensor else None
    in_names, out_names, out_avals, zero_shapes = [], [], [], []
    for alloc in nc.m.functions[0].allocations:
        if not isinstance(alloc, mybir.MemoryLocationSet):
            continue
        name = alloc.memorylocations[0].name
        if alloc.kind == "ExternalInput":
            if name != partition_name:
                in_names.append(name)
        elif alloc.kind == "ExternalOutput":
            out_names.append(name)
            shape = tuple(alloc.tensor_shape)
            dtype = mybir.dt.np(alloc.dtype)
            out_avals.append(jax.core.ShapedArray(shape, dtype))
            zero_shapes.append((shape, dtype))
    n_params = len(in_names)
    n_outs = len(out_avals)
    all_in_names = list(in_names) + list(out_names)
    if partition_name is not None:
        all_in_names.append(partition_name)

    def _body(*args):
        operands = list(args)
        if partition_name is not None:
            operands.append(partition_id_tensor())
        outs = _bass_exec_p.bind(
            *operands,
            out_avals=tuple(out_avals),
            in_names=tuple(all_in_names),
            out_names=tuple(out_names),
            lowering_input_output_aliases=(),
            sim_require_finite=True,
            sim_require_nnan=True,
            nc=nc,
        )
        return tuple(outs)

    devices = jax.devices()[:N_CORES]
    assert len(devices) >= N_CORES, f"need {N_CORES} devices, have {len(devices)}"
    mesh = Mesh(np.asarray(devices[:N_CORES]), ("core",))
    in_specs = (PartitionSpec("core"),) * (n_params + n_outs)
    out_specs = (PartitionSpec("core"),) * n_outs
    donate = tuple(range(n_params, n_params + n_outs))
    sharded = jax.jit(
        shard_map(
            _body, mesh=mesh, in_specs=in_specs, out_specs=out_specs, check_rep=False
        ),
        donate_argnums=donate,
        keep_unused=True,
    )

    def run(concat_inputs):
        args = [concat_inputs[name] for name in in_names]
        zeros = [
            np.zeros((N_CORES * s[0], *s[1:]), d) for (s, d) in zero_shapes
        ]
        out_arrs = sharded(*args, *zeros)
        out_arrs = [np.asarray(a) for a in out_arrs]
        return {name: out_arrs[i] for i, name in enumerate(out_names)}

    run.in_names = in_names
    run.out_names = out_names
    run.out_avals = out_avals
    run.zero_shapes = zero_shapes
    run.sharded = sharded
    run.mesh = mesh
    return run


def _plan(b_seq_len):
    """Mixed plan: per seq, floor(tiles/2) pair-groups + (tiles%2) singles.

    Returns (groups2, groups1, TG2, TG1, n_invalid):
      groups2: [(b, jg)] padded to 8*TG2 (pair jg covers seq tiles 2jg, 2jg+1)
      groups1: [(b, jt)] padded to 8*TG1 (single covers seq tile jt)
      n_invalid[b] = ntiles_b*128 - len_b  (tile-tail padding only)
    """
    import os
    pairs_only = bool(int(os.environ.get("KPLAN_PAIRS_ONLY", "0")))
    lens = [int(x) for x in b_seq_len]
    groups2, groups1 = [], []
    n_invalid = np.zeros(B, dtype=np.int64)
    for b, ln in enumerate(lens):
        ntiles = (ln + 127) // 128
        if pairs_only:
            ng = (ntiles + 1) // 2
            n_invalid[b] = ng * 256 - ln
            for jg in range(ng):
                groups2.append((b, jg))
            continue
        n_invalid[b] = ntiles * 128 - ln
        for jg in range(ntiles // 2):
            groups2.append((b, jg))
        if ntiles % 2:
            groups1.append((b, ntiles - 1))
    TG2 = (len(groups2) + N_CORES - 1) // N_CORES
    TG1 = (len(groups1) + N_CORES - 1) // N_CORES
    groups2 += [(-1, -1)] * (N_CORES * TG2 - len(groups2))
    groups1 += [(-1, -1)] * (N_CORES * TG1 - len(groups1))
    return groups2, groups1, TG2, TG1, n_invalid


def _pack_tiles(ent, kvb, lens, starts, csi, new_kv):
    """ent: [(b, j)] tile entries (b<0 = zero tile). Returns packed bytes
    [n, 128, 2*TW] uint8 (K^T bf16 | V fp8e3m4)."""
    import ml_dtypes

    bf = ml_dtypes.bfloat16
    e3 = ml_dtypes.float8_e3m4
    ent = np.array(ent, dtype=np.int64).reshape(-1, 2)
    n = len(ent)
    real = np.nonzero(ent[:, 0] >= 0)[0]
    eb, ej = ent[real, 0], ent[real, 1]

    rows = (starts[eb] + ej * 128)[:, None] + np.arange(128)[None, :]
    arr = kvb[rows]  # [nreal, 128, 2048] f32 gather
    nvalid = np.minimum(128, lens[eb] - ej * 128)
    mask = np.arange(128)[None, :] >= nvalid[:, None]
    arr[mask] = 0.0
    sel = np.nonzero((csi[eb] >= rows[:, 0]) & (csi[eb] < rows[:, 0] + 128))[0]
    arr[sel, (csi[eb[sel]] - rows[sel, 0])] = new_kv[eb[sel]]

    kt = (
        arr[:, :, :HKV * D].reshape(-1, 128, HKV, D)
        .transpose(0, 3, 2, 1).reshape(-1, 128, HKV * 128)
    )
    v = arr[:, :, HKV * D:]
    tiles = np.zeros((n, 128, 2 * TW), dtype=np.uint8)
    tiles[real, :, :2048] = kt.astype(bf).view(np.uint8)
    tiles[real, :, 2048:3072] = v.astype(e3).view(np.uint8)
    return tiles


def _pack(xq, xk, xv, kv_buffer, cur_select_index, start_index, b_seq_len,
          plan):
    import ml_dtypes

    bf = ml_dtypes.bfloat16
    groups2, groups1, TG2, TG1, _ = plan
    TGT = TG2 + TG1
    lens = np.asarray(b_seq_len, dtype=np.int64)
    starts = np.asarray(start_index, dtype=np.int64)
    csi = np.asarray(cur_select_index, dtype=np.int64)
    kvb = np.asarray(kv_buffer).reshape(-1, ROW)
    new_kv = np.concatenate(
        [np.asarray(xk)[:, 0], np.asarray(xv)[:, 0]], axis=1
    ).reshape(B, ROW)
    q_bf = np.asarray(xq)[:, 0].transpose(0, 2, 1).astype(bf)  # [B, D, HQ]

    ent2 = []
    for b, jg in groups2:
        for u in range(2):
            ent2.append((b, 2 * jg + u) if b >= 0 else (-1, -1))
    t2 = _pack_tiles(ent2, kvb, lens, starts, csi, new_kv)
    kv2_all = (
        t2.reshape(N_CORES * TG2, 2, 128, 2 * TW)
        .transpose(0, 2, 1, 3).reshape(N_CORES * TG2 * 128, 4 * TW)
        .view(bf)
    )
    out = {"kv2d": kv2_all}
    if TG1:
        t1 = _pack_tiles(groups1, kvb, lens, starts, csi, new_kv)
        out["kv1d"] = t1.reshape(N_CORES * TG1 * 128, 2 * TW).view(bf)

    qt_all = np.zeros((N_CORES, 128, 32 * TGT), dtype=bf)
    for c in range(N_CORES):
        for s in range(TG2):
            b = groups2[c * TG2 + s][0]
            if b >= 0:
                qt_all[c, :, 32 * s:32 * s + 32] = q_bf[b]
        for t in range(TG1):
            b = groups1[c * TG1 + t][0]
            if b >= 0:
                s = TG2 + t
                qt_all[c, :, 32 * s:32 * s + 32] = q_bf[b]
    out["qt"] = qt_all.reshape(N_CORES * 128, 32 * TGT)
    return out


def _combine(o_cat, l_cat, plan):
    groups2, groups1, TG2, TG1, n_invalid = plan
    TGT = TG2 + TG1
    acc = np.zeros((B, HQ, D), dtype=np.float64)
    lacc = np.zeros((B, HQ), dtype=np.float64)
    o_cat = np.asarray(o_cat).reshape(N_CORES, 16, 256 * TGT)
    l_cat = np.asarray(l_cat).reshape(N_CORES, 128, 2 * TGT)
    with np.errstate(invalid="ignore"):
        o_f = o_cat.astype(np.float64)

    def add_group(cc, s, b):
        for b2 in range(2):
            for c in range(4):
                hq = 16 * b2 + 4 * c
                Ob = o_f[cc][4 * c:4 * c + 4,
                             256 * s + 128 * b2:256 * s + 128 * (b2 + 1)]
                lb = l_cat[cc, 32 * c:32 * c + 4, 2 * s + b2]
                acc[b, hq:hq + 4] += Ob
                lacc[b, hq:hq + 4] += lb

    for i, (b, jg) in enumerate(groups2):
        if b >= 0:
            cc, s = divmod(i, TG2)
            add_group(cc, s, b)
    for i, (b, jt) in enumerate(groups1):
        if b >= 0:
            cc, t = divmod(i, TG1)
            add_group(cc, TG2 + t, b)
    lacc -= n_invalid[:, None]
    out = acc / lacc[:, :, None]
    return out.reshape(B, 1, HQ * D).astype(np.float32)


def get_compiled(TG2, TG1, niter=1):
    key = (TG2, TG1, niter)
    if key not in _COMPILED:
        nc = _build_program(TG2, TG1, niter)
        _COMPILED[key] = _make_runner(nc)
    return _COMPILED[key]


def kernel(xq, xk, xv, kv_buffer, cur_select_index, start_index, b_seq_len,
           max_actual_seq_len=None):
    plan = _plan(b_seq_len)
    inputs = _pack(xq, xk, xv, kv_buffer, cur_select_index, start_index,
                   b_seq_len, plan)
    run = get_compiled(plan[2], plan[3])
    outs = run(inputs)
    return _combine(outs["o"], outs["lout"], plan)


# revision 24
# speedup vs baseline: 1.0558x; 1.0088x over previous
"""Trainium2 Bass kernel v5: ragged GQA flash-decode attention.

Numerics (v3): K/Q/P bf16, V fp8 e3m4 (budget 2e-2; measures 1.679e-2).
Q (and P^T) are PE-stationary (4-col weight loads); K^T / V stream as the
moving operands. Scores+PV col-tile 4-way across PE quadrants
(tile_position); output strips must start 32-aligned (walrus). Host zeroes
invalid K/V slots (p=exp(0)=1 lands in l, 0 in O) and subtracts the exact
count from l. ACT exp's accum_out produces l for free.

Pipeline (v4, HW-trace driven):
  - For_i's per-iteration all-engine barrier costs ~14us, so `unroll`
    iterations are emitted per trip; across each barrier the pre-barrier
    body prefetches the next body's first kv groups into a fixed kv ring
    (streams during the compute tail) and defers its last wb chunk past
    the barrier (the next body flushes it).
  - qt loads once, outside the loop. The sync HWDGE queue carries ONLY kv
    loads; o/l writebacks ride gpsimd/SWDGE.
  - o is [16, 256*TGT] (real rows only); l is a separate 128-partition f32
    tensor so its writeback spreads across all SBUF AXI ports.

Packing (v5): no pair padding — sequences with an odd tile count get
`floor(tiles/2)` 2-tile groups plus ONE 1-tile group (second program phase),
cutting DMA bytes ~2.5%. Per core: TG2 pair-groups then TG1 singles.

Layouts (per core):
  kv2d [TG2*128, 3072] bf16-typed: pair-group g row p, col j*1536 + c;
       tile j: K^T bf16 at c in [0,1024) ([d=p, h*128+s]), V fp8e3m4
       (bitcast, bf16-col units) at c in [1024,1536) ([s=p, 64h + d/2]).
  kv1d [TG1*128, 1536]: same single-tile layout.
  qt   [128, 32*TGT] bf16: q for group g at cols 32g..32g+32, col = h*G+g.
  o    [16, 256*TGT] bf16: group g cols 256g+128*b2: head h=4*b2+c at row
       strip 4c..4c+4 (rows=[G]), cols = d.
  lout [128, 2*TGT] f32: accum_out sums; row 32c+g' (head 4*b2+c member g')
       at col 2g+b2.
"""

import math
from contextlib import ExitStack

import numpy as np

N_CORES = 8
B, HQ, HKV, D = 16, 32, 8, 128
G = HQ // HKV
ROW = 2 * HKV * D   # 2048 floats per kv_buffer row
TW = 1536           # bf16 cols per tile block (K 1024 + V-as-fp8 512)
GW = 2 * TW
SCALE = 1.0 / math.sqrt(D)

_COMPILED: dict = {}


def _build_program(TG2: int, TG1: int, niter: int = 1, *, kv_bufs=12,
                   psum_bufs=2, p_bufs=4, skew=2, wb_chunk=5, unroll=16,
                   prefetch=2, tp_bufs=2, sc_bufs=2, staggered=False):
    import concourse.mybir as mybir
    import concourse.tile as tile
    from concourse import bacc
    from concourse.masks import make_identity

    f32 = mybir.dt.float32
    bf16 = mybir.dt.bfloat16
    f8e3 = mybir.dt.float8e3
    nc = bacc.Bacc("TRN2", target_bir_lowering=False, debug=False, num_devices=N_CORES)

    TGT = TG2 + TG1
    kv2d = nc.dram_tensor("kv2d", [max(TG2, 1) * 128, GW], bf16,
                          kind="ExternalInput").ap()
    kv1d = None
    if TG1:
        kv1d = nc.dram_tensor("kv1d", [TG1 * 128, TW], bf16,
                              kind="ExternalInput").ap()
    qt = nc.dram_tensor("qt", [128, 32 * TGT], bf16, kind="ExternalInput").ap()
    o = nc.dram_tensor("o", [16, 256 * TGT], bf16, kind="ExternalOutput").ap()
    lout = nc.dram_tensor("lout", [128, 2 * TGT], f32, kind="ExternalOutput").ap()

    def gsrc(g):
        """DMA source AP and tile-width w for group g."""
        if g < TG2:
            return kv2d[g * 128:(g + 1) * 128, :], 2
        t = g - TG2
        return kv1d[t * 128:(t + 1) * 128, :], 1

    with tile.TileContext(nc) as tc, ExitStack() as ctx:
        kv_pool = ctx.enter_context(tc.tile_pool(name="kv", bufs=1))
        sc_pool = ctx.enter_context(
            tc.tile_pool(name="sc", bufs=sc_bufs, space="PSUM"))
        p_pools = [
            ctx.enter_context(tc.tile_pool(name=f"p{i}", bufs=p_bufs))
            for i in range(2)
        ]
        pts_pools = [
            ctx.enter_context(tc.tile_pool(name=f"pts{i}", bufs=p_bufs))
            for i in range(2)
        ]
        tp_ps_pool = ctx.enter_context(
            tc.tile_pool(name="tp", bufs=tp_bufs, space="PSUM"))
        o_pool = ctx.enter_context(tc.tile_pool(name="ops", bufs=psum_bufs, space="PSUM"))
        const_pool = ctx.enter_context(tc.tile_pool(name="const", bufs=1))
        io_pool = ctx.enter_context(tc.tile_pool(name="io", bufs=1))

        o_all = io_pool.tile([128, 256 * TGT], bf16)
        l_all = io_pool.tile([128, 2 * TGT], f32)

        ident = const_pool.tile([128, 128], bf16)
        make_identity(nc, ident[:])

        # qt is loop-invariant: load it once, before the iteration loop.
        qt_s = io_pool.tile([128, 32 * TGT], bf16)
        nc.sync.dma_start(qt_s[:], qt)

        # Fixed kv ring (explicit slices, not pool-rotated) so a body can
        # prefetch the NEXT body's first groups into the exact addresses that
        # body will read (cross-barrier identity).
        kv_ring = kv_pool.tile([128, kv_bufs * GW], bf16, name="kv_ring")
        kvt = [kv_ring[:, i * GW:(i + 1) * GW] for i in range(kv_bufs)]

        def kv_dma(g):
            src, w = gsrc(g)
            nc.sync.dma_start(kvt[g % kv_bufs][:, 0:w * TW], src)

        def v_ap(kv2, j, h):
            return kv2[:, TW * j + 1024 + 64 * h:
                       TW * j + 1024 + 64 * (h + 1)].bitcast(f8e3)

        def emit_pv(st):
            g, kv2, p, w = st
            # transpose P blocks on the PE (data loads as weights, identity
            # streams through), then one DVE copy per batch to SBUF
            pts = [pts_pools[i].tile([128, 128 * w], bf16, name=f"pts_{i}")
                   for i in range(2)]
            for i in range(2):
                tp = tp_ps_pool.tile([128, 128 * w], bf16, name=f"tp_{i}")
                for j in range(w):
                    nc.tensor.transpose(tp[:, 128 * j:128 * (j + 1)],
                                        p[i][:, 128 * j:128 * (j + 1)], ident[:])
                nc.vector.tensor_copy(pts[i][:], tp[:])
            O = o_pool.tile([128, 256], f32)
            for b2 in range(2):
                for j in range(w):
                    for c in range(4):
                        h = 4 * b2 + c
                        nc.tensor.matmul(
                            O[32 * c:32 * c + 4, 128 * b2:128 * b2 + 128],
                            pts[b2][:, 128 * j + 32 * c:128 * j + 32 * c + 4],
                            v_ap(kv2, j, h),
                            start=(j == 0), stop=(j == w - 1),
                            tile_position=(0, 32 * c),
                        )
            nc.vector.tensor_copy(o_all[:, 256 * g:256 * g + 256], O[:])
            if ((g + 1) % wb_chunk == 0 and g != TGT - 1) or \
                    (g == TGT - 1 and not emit_pv.defer_tail):
                wb_flush(emit_pv.wb_done, g + 1)
                emit_pv.wb_done = g + 1

        def wb_flush(g0, g1):
            c0, c1 = 256 * g0, 256 * g1
            for a in range(4):
                nc.gpsimd.dma_start(o[4 * a:4 * (a + 1), c0:c1],
                                    o_all[32 * a:32 * a + 4, c0:c1])
            nc.gpsimd.dma_start(lout[:, 2 * g0:2 * g1], l_all[:, 2 * g0:2 * g1])

        last_wb = TGT - ((TGT - 1) % wb_chunk + 1)  # start of the tail wb chunk

        def body(skip_pf=0, tail_pf=0, defer_tail=False, flush_prev=False):
            """One iteration. skip_pf: first N groups' kv DMAs were issued by
            the previous body (cross-barrier prefetch). tail_pf: issue the
            next body's first N kv DMAs after ours. defer_tail: leave the
            last wb chunk pending (the next body or the epilogue flushes it).
            flush_prev: write back the previous body's pending tail chunk."""
            emit_pv.wb_done = 0
            emit_pv.defer_tail = defer_tail
            if flush_prev:
                wb_flush(last_wb, TGT)
            pend = []
            for g in range(TGT):
                w = 2 if g < TG2 else 1
                kv2 = kvt[g % kv_bufs]
                if g >= skip_pf:
                    kv_dma(g)
                if g == TGT - 1:
                    for gp in range(tail_pf):
                        kv_dma(gp)
                kv2r = kv2[:, 0:w * TW].rearrange("p (j c) -> p j c", j=w)
                sc = sc_pool.tile([128, 256 * w], f32, name="sc")
                p = [p_pools[i].tile([128, 128 * w], bf16, name=f"p_{i}")
                     for i in range(2)]
                lg = l_all[:, 2 * g:2 * (g + 1)]
                for b2 in range(2):
                    for c in range(4):
                        h = 4 * b2 + c
                        nc.tensor.matmul(
                            sc[32 * c:32 * c + 4, 128 * w * b2:128 * w * (b2 + 1)],
                            qt_s[:, 32 * g + 4 * h:32 * g + 4 * h + 4],
                            kv2r[:, :, 128 * h:128 * (h + 1)],
                            start=True, stop=True,
                            tile_position=(0, 32 * c),
                        )
                    # exp for batch b2 starts while batch 1-b2 scores run
                    nc.scalar.activation(
                        p[b2][:],
                        sc[:, 128 * w * b2:128 * w * (b2 + 1)],
                        mybir.ActivationFunctionType.Exp,
                        scale=SCALE,
                        accum_out=lg[:, b2:b2 + 1],
                    )
                pend.append((g, kv2, p, w))
                if len(pend) > skew:
                    emit_pv(pend.pop(0))
            for st in pend:
                emit_pv(st)

        if niter > 1:
            # Unroll `unroll` iterations per For_i trip: the loop's all-engine
            # barrier (a full pipeline drain, ~14us) is paid once per trip.
            n_loop, n_rem = divmod(niter, unroll)
            PF = min(prefetch, TG2) if n_loop else 0
            if n_loop:
                for gp in range(PF):
                    kv_dma(gp)
                with tc.For_i(0, n_loop, 1, staggered_reset=staggered):
                    for u in range(unroll):
                        body(skip_pf=PF if u == 0 else 0,
                             tail_pf=PF if u == unroll - 1 else 0,
                             defer_tail=True, flush_prev=True)
            for r in range(n_rem):
                body(skip_pf=PF if r == 0 else 0,
                     defer_tail=True, flush_prev=(n_loop > 0 or r > 0))
            wb_flush(last_wb, TGT)
        else:
            body()

    nc.compile()
    return nc


def _make_runner(nc):
    """Persistent jitted SPMD runner for a compiled Bacc program (axon path)."""
    import jax
    import concourse.mybir as mybir
    from jax.experimental.shard_map import shard_map
    from jax.sharding import Mesh, PartitionSpec

    from concourse.bass2jax import (
        _bass_exec_p,
        install_neuronx_cc_hook,
        partition_id_tensor,
    )

    install_neuronx_cc_hook()

    partition_name = nc.partition_id_tensor.name if nc.partition_id_tensor else None
    in_names, out_names, out_avals, zero_shapes = [], [], [], []
    for alloc in nc.m.functions[0].allocations:
        if not isinstance(alloc, mybir.MemoryLocationSet):
            continue
        name = alloc.memorylocations[0].name
        if alloc.kind == "ExternalInput":
            if name != partition_name:
                in_names.append(name)
        elif alloc.kind == "ExternalOutput":
            out_names.append(name)
            shape = tuple(alloc.tensor_shape)
            dtype = mybir.dt.np(alloc.dtype)
            out_avals.append(jax.core.ShapedArray(shape, dtype))
            zero_shapes.append((shape, dtype))
    n_params = len(in_names)
    n_outs = len(out_avals)
    all_in_names = list(in_names) + list(out_names)
    if partition_name is not None:
        all_in_names.append(partition_name)

    def _body(*args):
        operands = list(args)
        if partition_name is not None:
            operands.append(partition_id_tensor())
        outs = _bass_exec_p.bind(
            *operands,
            out_avals=tuple(out_avals),
            in_names=tuple(all_in_names),
            out_names=tuple(out_names),
            lowering_input_output_aliases=(),
            sim_require_finite=True,
            sim_require_nnan=True,
            nc=nc,
        )
        return tuple(outs)

    devices = jax.devices()[:N_CORES]
    assert len(devices) >= N_CORES, f"need {N_CORES} devices, have {len(devices)}"
    mesh = Mesh(np.asarray(devices[:N_CORES]), ("core",))
    in_specs = (PartitionSpec("core"),) * (n_params + n_outs)
    out_specs = (PartitionSpec("core"),) * n_outs
    donate = tuple(range(n_params, n_params + n_outs))
    sharded = jax.jit(
        shard_map(
            _body, mesh=mesh, in_specs=in_specs, out_specs=out_specs, check_rep=False
        ),
        donate_argnums=donate,
        keep_unused=True,
    )

    def run(concat_inputs):
        args = [concat_inputs[name] for name in in_names]
        zeros = [
            np.zeros((N_CORES * s[0], *s[1:]), d) for (s, d) in zero_shapes
        ]
        out_arrs = sharded(*args, *zeros)
        out_arrs = [np.asarray(a) for a in out_arrs]
        return {name: out_arrs[i] for i, name in enumerate(out_names)}

    run.in_names = in_names
    run.out_names = out_names
    run.out_avals = out_avals
    run.zero_shapes = zero_shapes
    run.sharded = sharded
    run.mesh = mesh
    return run


def _plan(b_seq_len):
    """Mixed plan: per seq, floor(tiles/2) pair-groups + (tiles%2) singles.

    Returns (groups2, groups1, TG2, TG1, n_invalid):
      groups2: [(b, jg)] padded to 8*TG2 (pair jg covers seq tiles 2jg, 2jg+1)
      groups1: [(b, jt)] padded to 8*TG1 (single covers seq tile jt)
      n_invalid[b] = ntiles_b*128 - len_b  (tile-tail padding only)
    """
    import os
    # A/B showed the mixed singles phase costs ~2us despite 2.5% fewer
    # bytes (PSUM/pool size-class mixing); pairs-only is the default.
    pairs_only = bool(int(os.environ.get("KPLAN_PAIRS_ONLY", "1")))
    lens = [int(x) for x in b_seq_len]
    groups2, groups1 = [], []
    n_invalid = np.zeros(B, dtype=np.int64)
    for b, ln in enumerate(lens):
        ntiles = (ln + 127) // 128
        if pairs_only:
            ng = (ntiles + 1) // 2
            n_invalid[b] = ng * 256 - ln
            for jg in range(ng):
                groups2.append((b, jg))
            continue
        n_invalid[b] = ntiles * 128 - ln
        for jg in range(ntiles // 2):
            groups2.append((b, jg))
        if ntiles % 2:
            groups1.append((b, ntiles - 1))
    TG2 = (len(groups2) + N_CORES - 1) // N_CORES
    TG1 = (len(groups1) + N_CORES - 1) // N_CORES
    groups2 += [(-1, -1)] * (N_CORES * TG2 - len(groups2))
    groups1 += [(-1, -1)] * (N_CORES * TG1 - len(groups1))
    return groups2, groups1, TG2, TG1, n_invalid


def _pack_tiles(ent, kvb, lens, starts, csi, new_kv):
    """ent: [(b, j)] tile entries (b<0 = zero tile). Returns packed bytes
    [n, 128, 2*TW] uint8 (K^T bf16 | V fp8e3m4)."""
    import ml_dtypes

    bf = ml_dtypes.bfloat16
    e3 = ml_dtypes.float8_e3m4
    ent = np.array(ent, dtype=np.int64).reshape(-1, 2)
    n = len(ent)
    real = np.nonzero(ent[:, 0] >= 0)[0]
    eb, ej = ent[real, 0], ent[real, 1]

    rows = (starts[eb] + ej * 128)[:, None] + np.arange(128)[None, :]
    arr = kvb[rows]  # [nreal, 128, 2048] f32 gather
    nvalid = np.minimum(128, lens[eb] - ej * 128)
    mask = np.arange(128)[None, :] >= nvalid[:, None]
    arr[mask] = 0.0
    sel = np.nonzero((csi[eb] >= rows[:, 0]) & (csi[eb] < rows[:, 0] + 128))[0]
    arr[sel, (csi[eb[sel]] - rows[sel, 0])] = new_kv[eb[sel]]

    kt = (
        arr[:, :, :HKV * D].reshape(-1, 128, HKV, D)
        .transpose(0, 3, 2, 1).reshape(-1, 128, HKV * 128)
    )
    v = arr[:, :, HKV * D:]
    tiles = np.zeros((n, 128, 2 * TW), dtype=np.uint8)
    tiles[real, :, :2048] = kt.astype(bf).view(np.uint8)
    tiles[real, :, 2048:3072] = v.astype(e3).view(np.uint8)
    return tiles


def _pack(xq, xk, xv, kv_buffer, cur_select_index, start_index, b_seq_len,
          plan):
    import ml_dtypes

    bf = ml_dtypes.bfloat16
    groups2, groups1, TG2, TG1, _ = plan
    TGT = TG2 + TG1
    lens = np.asarray(b_seq_len, dtype=np.int64)
    starts = np.asarray(start_index, dtype=np.int64)
    csi = np.asarray(cur_select_index, dtype=np.int64)
    kvb = np.asarray(kv_buffer).reshape(-1, ROW)
    new_kv = np.concatenate(
        [np.asarray(xk)[:, 0], np.asarray(xv)[:, 0]], axis=1
    ).reshape(B, ROW)
    q_bf = np.asarray(xq)[:, 0].transpose(0, 2, 1).astype(bf)  # [B, D, HQ]

    ent2 = []
    for b, jg in groups2:
        for u in range(2):
            ent2.append((b, 2 * jg + u) if b >= 0 else (-1, -1))
    t2 = _pack_tiles(ent2, kvb, lens, starts, csi, new_kv)
    kv2_all = (
        t2.reshape(N_CORES * TG2, 2, 128, 2 * TW)
        .transpose(0, 2, 1, 3).reshape(N_CORES * TG2 * 128, 4 * TW)
        .view(bf)
    )
    out = {"kv2d": kv2_all}
    if TG1:
        t1 = _pack_tiles(groups1, kvb, lens, starts, csi, new_kv)
        out["kv1d"] = t1.reshape(N_CORES * TG1 * 128, 2 * TW).view(bf)

    qt_all = np.zeros((N_CORES, 128, 32 * TGT), dtype=bf)
    for c in range(N_CORES):
        for s in range(TG2):
            b = groups2[c * TG2 + s][0]
            if b >= 0:
                qt_all[c, :, 32 * s:32 * s + 32] = q_bf[b]
        for t in range(TG1):
            b = groups1[c * TG1 + t][0]
            if b >= 0:
                s = TG2 + t
                qt_all[c, :, 32 * s:32 * s + 32] = q_bf[b]
    out["qt"] = qt_all.reshape(N_CORES * 128, 32 * TGT)
    return out


def _combine(o_cat, l_cat, plan):
    groups2, groups1, TG2, TG1, n_invalid = plan
    TGT = TG2 + TG1
    acc = np.zeros((B, HQ, D), dtype=np.float64)
    lacc = np.zeros((B, HQ), dtype=np.float64)
    o_cat = np.asarray(o_cat).reshape(N_CORES, 16, 256 * TGT)
    l_cat = np.asarray(l_cat).reshape(N_CORES, 128, 2 * TGT)
    with np.errstate(invalid="ignore"):
        o_f = o_cat.astype(np.float64)

    def add_group(cc, s, b):
        for b2 in range(2):
            for c in range(4):
                hq = 16 * b2 + 4 * c
                Ob = o_f[cc][4 * c:4 * c + 4,
                             256 * s + 128 * b2:256 * s + 128 * (b2 + 1)]
                lb = l_cat[cc, 32 * c:32 * c + 4, 2 * s + b2]
                acc[b, hq:hq + 4] += Ob
                lacc[b, hq:hq + 4] += lb

    for i, (b, jg) in enumerate(groups2):
        if b >= 0:
            cc, s = divmod(i, TG2)
            add_group(cc, s, b)
    for i, (b, jt) in enumerate(groups1):
        if b >= 0:
            cc, t = divmod(i, TG1)
            add_group(cc, TG2 + t, b)
    lacc -= n_invalid[:, None]
    out = acc / lacc[:, :, None]
    return out.reshape(B, 1, HQ * D).astype(np.float32)


def get_compiled(TG2, TG1, niter=1):
    key = (TG2, TG1, niter)
    if key not in _COMPILED:
        nc = _build_program(TG2, TG1, niter)
        _COMPILED[key] = _make_runner(nc)
    return _COMPILED[key]


def kernel(xq, xk, xv, kv_buffer, cur_select_index, start_index, b_seq_len,
           max_actual_seq_len=None):
    plan = _plan(b_seq_len)
    inputs = _pack(xq, xk, xv, kv_buffer, cur_select_index, start_index,
                   b_seq_len, plan)
    run = get_compiled(plan[2], plan[3])
    outs = run(inputs)
    return _combine(outs["o"], outs["lout"], plan)
